# revision 16
# baseline (speedup 1.0000x reference)
"""MoE layer (8 experts, top-2 routing + shared expert) on 8 Trainium2 cores.

Strategy (expert parallelism per the sharding hint):
  - Host computes the router and dispatches: core e gets the tokens routed
    to expert e plus a 1/8 data-parallel slice of all tokens for the shared
    expert. Host scatter-adds the outputs back (combine).
  - Each core computes y = (silu(x@Wg.T) * (x@Wu.T)) @ Wd.T for its expert
    tokens, then for the shared slice, scaled by combine weights.

Numerics: fp8e4m3 DoubleRow matmuls (256-deep contraction, 0.5 cyc/row)
with an exact two-term (hi+lo) split of every operand. Each 256-row slice
needs 3 products (hi*hi + hi*lo + lo*hi; lo*lo ~ 7e-4 dropped) -> 4/3 of
fp16 PE throughput with end-to-end error ~4e-3 (verified vs fp32 ref).

Real-HW scheduling: the PE pays a ~256-cycle stationary (weight) load per
matmul unless consecutive matmuls share the stationary AP; the loads then
pipeline behind long runs of moving-operand streams. So each phase keeps
its whole token set resident in SBUF and, for every stationary tile, streams
all token slabs (gemm1: 18/9 streams per load) or all d-slabs (gemm2: 16/8
streams per load, 8 PSUM slabs live).

Scaling: host pre-scales Wg*32, Wu*8, Wd*32 (keeps hi and lo terms out of
fp8 flush range; residuals live in e4m3 subnormals which this HW honors
exactly). Device: silu16 = Silu(pg/32), t16 = silu16*pu (= 8a), ahi =
fp8(t16), alo = fp8(t16-ahi). 1/(8*32) is folded into the combine weight.

Layouts per core (fp8 except cw/y):
  xh_e/xl_e [128, 8, 2, C]   xh_s/xl_s [128, 8, 2, S0]   (p, kpair, i, tok)
  wg/wu hi+lo [128, 11, 8, 2, 128]  (p, ht, kpair, i, m) -- streamed per ht
  wd hi+lo    [128, 6, 2, 2048]     (p, hpair, i, dcol), jh=5 packs
              (Wdhi10, Wdhi10) / (Wdlo10, 0) so stationary (ahi10, alo10)
              covers all three tile-10 products
  cw [128, TT/128] f32 (weight/256), y [TT, 2048] fp16
"""

import numpy as np
import ml_dtypes

import concourse.mybir as mybir
import concourse.tile as tile
from concourse import bacc
from concourse.bass import ds
from concourse.bass_utils import run_bass_kernel_spmd

P = 128
D = 2048
H = 1408
E = 8
TOP_K = 2
KJ = D // 256      # 8 k-pairs
NHT = H // P       # 11 h-tiles
JH = 6             # h-pairs incl. packed tail
DT8 = mybir.dt.float8e4
F16 = mybir.dt.float16
F32 = mybir.dt.float32
NP8 = ml_dtypes.float8_e4m3
DR = mybir.MatmulPerfMode.DoubleRow
Silu = mybir.ActivationFunctionType.Silu
Copy = mybir.ActivationFunctionType.Copy

SWG, SWU, SWD = 32.0, 8.0, 32.0
OSCALE = 1.0 / (SWU * SWD)   # folded into cw on host


def _slabs(count):
    """Token slabs of 256 (+ trailing 128)."""
    out = []
    pos = 0
    while count - pos >= 256:
        out.append((pos, 256))
        pos += 256
    if count - pos >= P:
        out.append((pos, P))
        pos += P
    assert pos == count
    return out


def build_kernel(C, S0, repeat=1, wsb=2, psb=8, ob=2, t16b=2, xpieces=3,
                 drain_split=True):
    TT = C + S0
    assert C % P == 0 and S0 % P == 0

    nc = bacc.Bacc(
        "TRN2",
        target_bir_lowering=False,
        debug=False,
        enable_asserts=False,
        num_devices=8,
    )

    xd = {}
    for pref, T in (("e", C), ("s", S0)):
        xd[pref] = (
            nc.dram_tensor(f"xh_{pref}", [P, KJ, 2, T], DT8, kind="ExternalInput").ap(),
            nc.dram_tensor(f"xl_{pref}", [P, KJ, 2, T], DT8, kind="ExternalInput").ap(),
        )
    wts = {}
    for pref in ("e", "s"):
        wts[pref] = {
            nm: nc.dram_tensor(f"{nm}_{pref}",
                               [P, NHT, KJ, 2, P] if nm[1] != "d" else [P, JH, 2, D],
                               DT8, kind="ExternalInput").ap()
            for nm in ("wgh", "wgl", "wuh", "wul", "wdh", "wdl")
        }
    cw = nc.dram_tensor("cw", [P, TT // P], F32, kind="ExternalInput").ap()
    y = nc.dram_tensor("y", [TT, D], F16, kind="ExternalOutput").ap()
    y_r = y.rearrange("(g p) d -> p g d", p=P)

    phases = [("e", 0, C), ("s", C, S0)] * repeat

    with tile.TileContext(nc) as tc:
        with (
            tc.tile_pool(name="xhp", bufs=1) as xhp,
            tc.tile_pool(name="xlp", bufs=1) as xlp,
            tc.tile_pool(name="wg1p", bufs=wsb) as wg1p,   # hi, gemm1 (g/u share)
            tc.tile_pool(name="wl1p", bufs=wsb) as wl1p,   # lo, gemm1
            tc.tile_pool(name="wdhp", bufs=1) as wdhp,
            tc.tile_pool(name="wdlp", bufs=1) as wdlp,
            tc.tile_pool(name="athp", bufs=1) as athp,
            tc.tile_pool(name="atlp", bufs=1) as atlp,
            tc.tile_pool(name="silp", bufs=1) as silp,
            tc.tile_pool(name="t16p", bufs=t16b) as t16p,
            tc.tile_pool(name="op", bufs=ob) as opool,
            tc.tile_pool(name="cp", bufs=1) as cpool,
            tc.tile_pool(name="ps", bufs=psb, space="PSUM") as psp,
        ):
            cw_sb = cpool.tile([P, TT // P], F32)
            nc.sync.dma_start(cw_sb[:], cw)

            for pref, base, T in phases:
                wt = wts[pref]
                slabs = _slabs(T)
                K = len(slabs)

                # ---- x for the whole phase, split DMAs so slab 0 lands fast
                # (single buffer shared across phases: s-phase x loads during
                # the e-phase gemm2 window)
                xh_sb = xhp.tile([P, KJ, 2, T], DT8, tag="xh", name="xh_sb")
                xl_sb = xlp.tile([P, KJ, 2, T], DT8, tag="xl", name="xl_sb")
                bounds = [T * i // xpieces // P * P for i in range(xpieces + 1)]
                bounds[-1] = T
                for b0, b1 in zip(bounds[:-1], bounds[1:]):
                    if b1 > b0:
                        nc.sync.dma_start(xh_sb[:, :, :, b0:b1], xd[pref][0][:, :, :, b0:b1])
                for b0, b1 in zip(bounds[:-1], bounds[1:]):
                    if b1 > b0:
                        nc.sync.dma_start(xl_sb[:, :, :, b0:b1], xd[pref][1][:, :, :, b0:b1])

                # ---- gemm2 weights: DMA from the Pool engine's queue so they
                # run during gemm1 instead of queuing behind the gated per-ht
                # gemm1 weight DMAs on SP
                wdh_sb = wdhp.tile([P, JH, 2, D], DT8, tag="wdh", name="wdh_sb")
                wdl_sb = wdlp.tile([P, JH, 2, D], DT8, tag="wdl", name="wdl_sb")
                nc.gpsimd.dma_start(wdh_sb[:], wt["wdh"])
                nc.gpsimd.dma_start(wdl_sb[:], wt["wdl"])

                ath = athp.tile([P, JH, 2, T], DT8, tag="ath", name="ath")
                atl = atlp.tile([P, JH - 1, 2, T], DT8, tag="atl", name="atl")

                # ---- gemm1
                for ht in range(NHT):
                    sil = silp.tile([P, T], F16, tag="sil", name="sil")
                    t16 = t16p.tile([P, T], F16, tag="t16", name="t16")
                    for path in ("g", "u"):
                        whi = wg1p.tile([P, KJ, 2, P], DT8, tag="w1h", name="whi_sb")
                        wlo = wl1p.tile([P, KJ, 2, P], DT8, tag="w1l", name="wlo_sb")
                        hi_nm = "wgh" if path == "g" else "wuh"
                        lo_nm = "wgl" if path == "g" else "wul"
                        nc.sync.dma_start(whi[:], wt[hi_nm][:, ht])
                        nc.sync.dma_start(wlo[:], wt[lo_nm][:, ht])
                        # PSUM banks are 2KB: one [P,512] tile per 512 tokens,
                        # 256-slab matmuls address halves
                        nbank = (T + 511) // 512
                        pt = [psp.tile([P, 512], F32, tag="ps", name="pt")
                              for _ in range(nbank)]

                        def pslab(off, w):
                            return pt[off // 512][:, off % 512:off % 512 + w]

                        for j in range(KJ):
                            st = whi[:, j]
                            for off, w in slabs:
                                nc.tensor.matmul(pslab(off, w), st,
                                                 xh_sb[:, j, :, ds(off, w)],
                                                 start=(j == 0), stop=False,
                                                 perf_mode=DR, skip_group_check=True)
                            for off, w in slabs:
                                nc.tensor.matmul(pslab(off, w), st,
                                                 xl_sb[:, j, :, ds(off, w)],
                                                 start=False, stop=False,
                                                 perf_mode=DR, skip_group_check=True)
                            st = wlo[:, j]
                            for off, w in slabs:
                                nc.tensor.matmul(pslab(off, w), st,
                                                 xh_sb[:, j, :, ds(off, w)],
                                                 start=False, stop=(j == KJ - 1),
                                                 perf_mode=DR, skip_group_check=True)
                        if path == "g":
                            for off, w in slabs:
                                nc.scalar.activation(sil[:, ds(off, w)], pslab(off, w),
                                                     Silu, scale=1.0 / SWG)
                        else:
                            hi_dst = ath[:, ht // 2, ht % 2, :]
                            lo_dst = ath[:, 5, 1, :] if ht == 10 else atl[:, ht // 2, ht % 2, :]
                            for off, w in slabs:
                                nc.vector.tensor_tensor(t16[:, ds(off, w)],
                                                        sil[:, ds(off, w)], pslab(off, w),
                                                        mybir.AluOpType.mult)
                                nc.scalar.activation(hi_dst[:, ds(off, w)],
                                                     t16[:, ds(off, w)], Copy)
                                nc.vector.tensor_tensor(lo_dst[:, ds(off, w)],
                                                        t16[:, ds(off, w)],
                                                        hi_dst[:, ds(off, w)],
                                                        mybir.AluOpType.subtract)

                # ---- gemm2
                for tg in range(T // P):
                    gg = (base + tg * P) // P
                    tok = ds(tg * P, P)
                    out_sb = opool.tile([P, D], F16, tag="o", name="out_sb")
                    py = [psp.tile([P, 512], F32, tag="ps", name="py") for _ in range(4)]

                    def pyd(db):
                        return py[db // 2][:, (db % 2) * 256:(db % 2) * 256 + 256]

                    for jh in range(JH):
                        la = ath[:, jh, :, tok]
                        for db in range(8):
                            nc.tensor.matmul(pyd(db), la,
                                             wdh_sb[:, jh, :, ds(db * 256, 256)],
                                             start=(jh == 0), stop=False,
                                             perf_mode=DR, skip_group_check=True)
                        for db in range(8):
                            nc.tensor.matmul(pyd(db), la,
                                             wdl_sb[:, jh, :, ds(db * 256, 256)],
                                             start=False, stop=(jh == JH - 1),
                                             perf_mode=DR, skip_group_check=True)
                        if jh < JH - 1:
                            la = atl[:, jh, :, tok]
                            for db in range(8):
                                nc.tensor.matmul(pyd(db), la,
                                                 wdh_sb[:, jh, :, ds(db * 256, 256)],
                                                 start=False, stop=False,
                                                 perf_mode=DR, skip_group_check=True)
                    for db in range(8):
                        dst = out_sb[:, ds(db * 256, 256)]
                        if drain_split and db % 2 == 0:
                            nc.scalar.activation(dst, pyd(db), Copy,
                                                 scale=cw_sb[:, gg:gg + 1])
                        else:
                            nc.vector.tensor_scalar_mul(dst, pyd(db),
                                                        cw_sb[:, gg:gg + 1])
                    # y stores from the scalar queue: keeps SP free for the
                    # next phase's x/weight DMAs
                    nc.scalar.dma_start(y_r[:, gg, :], out_sb[:])

    nc.compile()
    return nc


def _route(x_flat, gate_w, expert_bias):
    """Replicate the reference router in numpy (fp32)."""
    N = x_flat.shape[0]
    logits = x_flat @ gate_w.T                       # [N, E]
    m = logits.max(-1, keepdims=True)
    p = np.exp(logits - m)
    p /= p.sum(-1, keepdims=True)
    biased = logits + expert_bias
    rows = np.arange(N)
    i1 = biased.argmax(-1)
    b2 = biased.copy()
    b2[rows, i1] = -np.inf
    i2 = b2.argmax(-1)
    w1 = p[rows, i1]
    w2 = p[rows, i2]
    s = w1 + w2
    return i1, i2, w1 / s, w2 / s


def _split8(v, s):
    vs = (v * s).astype(np.float32)
    hi = vs.astype(NP8)
    lo = (vs - hi.astype(np.float32)).astype(NP8)
    return hi, lo


def _pack_g(w8):
    """[H, D] fp8 -> [128, 11, 8, 2, 128] (p, ht, j, i, m)."""
    return np.ascontiguousarray(
        w8.reshape(NHT, P, KJ, 2, P).transpose(4, 0, 2, 3, 1))


def _pack_wd(Wd):
    """Wd [D, H] fp32 -> (wdh, wdl) [128, 6, 2, D] with the jh=5 packing."""
    hi, lo = _split8(Wd, SWD)                       # [D, H] fp8
    hif = np.zeros((D, JH * 2 * P), dtype=NP8)
    lof = np.zeros((D, JH * 2 * P), dtype=NP8)
    hif[:, :H] = hi
    lof[:, :H] = lo
    hif[:, H:H + P] = hi[:, 10 * P:]                # (5,1) dup of Wdhi10
    # lof[:, H:H+P] stays 0
    pack = lambda a: np.ascontiguousarray(
        a.reshape(D, JH, 2, P).transpose(3, 1, 2, 0))
    return pack(hif), pack(lof)


def _pack_x(x8):
    """[T, D] fp8 -> [128, 8, 2, T] (p, j, i, tok)."""
    T = x8.shape[0]
    return np.ascontiguousarray(x8.reshape(T, KJ, 2, P).transpose(3, 1, 2, 0))


def _prepare(inputs):
    x = np.asarray(inputs["x"], dtype=np.float32)
    B, S_, D_ = x.shape
    assert D_ == D
    x_flat = x.reshape(-1, D)
    N = x_flat.shape[0]
    S0 = N // 8

    i1, i2, w1, w2 = _route(
        x_flat,
        np.asarray(inputs["gate_w"], dtype=np.float32),
        np.asarray(inputs["expert_bias"], dtype=np.float32),
    )

    idx_lists, w_lists = [], []
    for e in range(E):
        m1 = i1 == e
        m2 = i2 == e
        idx = np.nonzero(m1 | m2)[0]
        w = np.where(m1[idx], w1[idx], w2[idx]).astype(np.float32)
        idx_lists.append(idx)
        w_lists.append(w)

    maxc = max(len(ix) for ix in idx_lists)
    C = ((maxc + P - 1) // P) * P
    TT = C + S0

    xhi = x_flat.astype(NP8)
    xlo = (x_flat - xhi.astype(np.float32)).astype(NP8)

    Wg = np.asarray(inputs["Wg"], dtype=np.float32)
    Wu = np.asarray(inputs["Wu"], dtype=np.float32)
    Wd = np.asarray(inputs["Wd"], dtype=np.float32)

    shared = {}
    gh, gl = _split8(np.asarray(inputs["Ws_g"], np.float32), SWG)
    shared["wgh_s"], shared["wgl_s"] = _pack_g(gh), _pack_g(gl)
    uh, ul = _split8(np.asarray(inputs["Ws_u"], np.float32), SWU)
    shared["wuh_s"], shared["wul_s"] = _pack_g(uh), _pack_g(ul)
    shared["wdh_s"], shared["wdl_s"] = _pack_wd(np.asarray(inputs["Ws_d"], np.float32))

    in_maps = []
    idx_pad = np.empty((E, C), dtype=np.int64)
    for e in range(E):
        idx = idx_lists[e]
        pad = np.full(C - len(idx), N, dtype=np.int64)  # N -> dummy row
        idx_pad[e] = np.concatenate([idx, pad])
        gather_idx = np.concatenate([idx, np.zeros(C - len(idx), np.int64)])
        srange = np.arange(e * S0, (e + 1) * S0)

        cwv = np.full(TT, OSCALE, dtype=np.float32)
        cwv[:len(idx)] = w_lists[e] * OSCALE
        cwv[len(idx):C] = 0.0
        cwv = np.ascontiguousarray(cwv.reshape(TT // P, P).T)

        gh, gl = _split8(Wg[e], SWG)
        uh, ul = _split8(Wu[e], SWU)
        wdh, wdl = _pack_wd(Wd[e])
        m = {
            "xh_e": _pack_x(xhi[gather_idx]), "xl_e": _pack_x(xlo[gather_idx]),
            "xh_s": _pack_x(xhi[srange]), "xl_s": _pack_x(xlo[srange]),
            "wgh_e": _pack_g(gh), "wgl_e": _pack_g(gl),
            "wuh_e": _pack_g(uh), "wul_e": _pack_g(ul),
            "wdh_e": wdh, "wdl_e": wdl,
            "cw": cwv,
            **shared,
        }
        in_maps.append(m)
    return x, in_maps, idx_pad, C, S0, N


def _combine(x_shape, results, idx_pad, C, S0, N):
    acc = np.zeros((N + 1, D), dtype=np.float32)
    for e in range(E):
        ye = results[e]["y"].astype(np.float32)
        acc[idx_pad[e]] += ye[:C]
        acc[e * S0:(e + 1) * S0] += ye[C:]
    return acc[:N].reshape(x_shape)


def kernel(**inputs) -> np.ndarray:
    x, in_maps, idx_pad, C, S0, N = _prepare(inputs)
    nc = build_kernel(C, S0)
    res = run_bass_kernel_spmd(nc, in_maps, core_ids=list(range(8)))
    return _combine(x.shape, [res.results[e] for e in range(E)], idx_pad, C, S0, N)


# revision 23
# speedup vs baseline: 1.2923x; 1.2923x over previous
"""MoE layer (8 experts, top-2 routing + shared expert) on 8 Trainium2 cores.

Strategy (expert parallelism per the sharding hint):
  - Host computes the router (logits -> softmax -> top-2 -> combine weights)
    and *dispatches*: core e receives the tokens routed to expert e (gathered,
    transposed to [D, C] layout, fp16) plus a 1/8 data-parallel slice of all
    tokens for the shared expert.
  - Each core runs one Bass/Tile kernel computing, for its token set,
      y = (silu(x @ Wg.T) * (x @ Wu.T)) @ Wd.T   (scaled by combine weight)
    for its expert's weights, then the same with the shared-expert weights.
    All matmuls are fp16 with fp32 PSUM accumulation.
  - Host *combines*: scatter-adds the per-expert outputs and the shared
    outputs back into the full [N, D] result.

Device layout per core (SPMD, one NEFF):
  xt  [D, TT]  fp16   tokens on the free dim, D on partitions (16 k-tiles)
  wg,wu [D, H] fp16   expert-then-shared weight loads (H on free dim)
  wd  [H, D]   fp16
  cw  [128, TT/128] f32  per-token combine weight, pre-grouped on host so
                         the DMA is contiguous (1.0 for the shared slice)
  y   [TT, D]  f32    output, tokens on partitions at write time

Pipeline per 512-token chunk: 2*11*16 matmuls produce g,u in PSUM per
128-row H tile; ScalarE applies Silu, VectorE multiplies into an fp16 act
tile [H, chunk]; 4x4x11 matmuls then contract act.T @ WdT into [128 tokens,
512 D] PSUM tiles, which VectorE scales by cw and DMAs out.
"""

import numpy as np

import concourse.mybir as mybir
import concourse.tile as tile
from concourse import bacc
from concourse.bass import ds
from concourse.bass_utils import run_bass_kernel_spmd

P = 128
D = 2048
H = 1408
E = 8
TOP_K = 2
KD = D // P   # 16
KH = H // P   # 11
DT16 = mybir.dt.float16  # fp16: same PE rate as bf16, 8x the mantissa precision
F32 = mybir.dt.float32


def _chunks(count, base):
    """Split `count` tokens (multiple of 128) into chunks of 512 then 128."""
    out = []
    pos = 0
    while count - pos >= 512:
        out.append((base + pos, 512))
        pos += 512
    while count - pos >= P:
        out.append((base + pos, P))
        pos += P
    assert pos == count
    return out


def build_kernel(C, S, repeat=1, xb=2, ab=2, ob=2, pgu=3, pyb=2, wd_late=False, ysplit=False, tail_first=False, psg=None, psu=None, bulk_dma=True, store_eng="sync", y16=True, hsplit=4, gu_interleave=True, wd_eng="sync", x_eng="sync"):
    """Build the SPMD Bass module for C expert tokens + S shared tokens."""
    TT = C + S
    assert C % P == 0 and S % P == 0

    nc = bacc.Bacc(
        "TRN2",
        target_bir_lowering=False,
        debug=False,
        enable_asserts=False,
        num_devices=8,
    )

    xt = nc.dram_tensor("xt", [D, TT], DT16, kind="ExternalInput").ap()
    wts = {}
    for pref in ("e", "s"):
        wts[pref] = (
            nc.dram_tensor(f"wg_{pref}", [D, H], DT16, kind="ExternalInput").ap(),
            nc.dram_tensor(f"wu_{pref}", [D, H], DT16, kind="ExternalInput").ap(),
            nc.dram_tensor(f"wd_{pref}", [H, D], DT16, kind="ExternalInput").ap(),
        )
    cw = nc.dram_tensor("cw", [P, TT // P], F32, kind="ExternalInput").ap()
    y = nc.dram_tensor("y", [TT, D], DT16 if y16 else F32, kind="ExternalOutput").ap()

    xt_r = xt.rearrange("(ko p) t -> p ko t", p=P)     # [128, 16, TT]
    y_r = y.rearrange("(g p) d -> p g d", p=P)         # [128, TT/128, 2048]
    cw_r = cw  # already [128, TT/128] host-transposed

    phases = [("e", 0, C), ("s", C, S)]

    with tile.TileContext(nc) as tc:
        with (
            tc.tile_pool(name="wgp", bufs=1) as wgp,
            tc.tile_pool(name="wup", bufs=1) as wup,
            tc.tile_pool(name="wdp", bufs=1) as wdp,
            tc.tile_pool(name="xp", bufs=xb) as xp,
            tc.tile_pool(name="ap", bufs=ab) as apool,
            tc.tile_pool(name="op", bufs=ob) as opool,
            tc.tile_pool(name="cp", bufs=1) as cpool,
            tc.tile_pool(name="psgu", bufs=pgu, space="PSUM") as psgu,
            tc.tile_pool(name="psgu2", bufs=(psu or pgu), space="PSUM") as psgu2,
            tc.tile_pool(name="psy", bufs=pyb, space="PSUM") as psy,
        ):
            cw_sb = cpool.tile([P, TT // P], F32)
            nc.sync.dma_start(cw_sb[:], cw_r)

            for pref, base, count in phases * repeat:
                if count == 0:
                    continue
                wg_d, wu_d, wd_d = wts[pref]
                wg_sb = wgp.tile([P, KD, H], DT16, tag="wg")
                wu_sb = wup.tile([P, KD, H], DT16, tag="wu")
                wg_rr = wg_d.rearrange("(ko p) h -> p ko h", p=P)
                wu_rr = wu_d.rearrange("(ko p) h -> p ko h", p=P)
                chunk_list = _chunks(count, base)
                if tail_first:
                    chunk_list = chunk_list[::-1]
                # bulk DMAs: a single large dma_start fans out across several
                # HW-DGE queues on real hardware (measured ~120us faster than
                # k-tile-split DMAs, even though the cost model says otherwise)
                start0, w0 = chunk_list[0]
                x0_sb = xp.tile([P, KD, 512], DT16, tag="x", name="x0_sb")[:, :, :w0]
                if bulk_dma:
                    nc.sync.dma_start(x0_sb[:], xt_r[:, :, ds(start0, w0)])
                    if hsplit > 1:
                        # split along H so early h-tiles' weights land first;
                        # pieces stay >=1.4MB for multi-queue DMA fanout
                        bounds = [H * i // hsplit for i in range(hsplit + 1)]
                        bounds = [(b // P) * P for b in bounds]
                        bounds[-1] = H
                        for h0, h1 in zip(bounds[:-1], bounds[1:]):
                            nc.sync.dma_start(wg_sb[:, :, h0:h1], wg_rr[:, :, h0:h1])
                            nc.sync.dma_start(wu_sb[:, :, h0:h1], wu_rr[:, :, h0:h1])
                    else:
                        nc.sync.dma_start(wg_sb[:], wg_rr)
                        nc.sync.dma_start(wu_sb[:], wu_rr)
                else:
                    for k in range(KD):
                        nc.sync.dma_start(x0_sb[:, k, :], xt_r[:, k, ds(start0, w0)])
                        nc.sync.dma_start(wg_sb[:, k, :], wg_rr[:, k, :])
                        nc.sync.dma_start(wu_sb[:, k, :], wu_rr[:, k, :])
                wd_sb = wdp.tile([P, KH, D], DT16, tag="wd")
                wd_rr = wd_d.rearrange("(ho p) d -> p ho d", p=P)
                wde = nc.gpsimd if wd_eng == "pool" else nc.sync
                if bulk_dma:
                    wde.dma_start(wd_sb[:], wd_rr)
                elif not wd_late:
                    for h in range(KH):
                        nc.sync.dma_start(wd_sb[:, h, :], wd_rr[:, h, :])

                for ci, (start, w) in enumerate(chunk_list):
                    if wd_late and ci == 1:
                        for h in range(KH):
                            nc.sync.dma_start(wd_sb[:, h, :], wd_rr[:, h, :])
                    if ci == 0:
                        x_sb = x0_sb
                    else:
                        x_sb = xp.tile([P, KD, 512], DT16, tag="x", name="x_sb")[:, :, :w]
                        xe = nc.gpsimd if x_eng == "pool" else nc.sync
                        if bulk_dma:
                            xe.dma_start(x_sb, xt_r[:, :, ds(start, w)])
                        else:
                            for k in range(KD):
                                nc.sync.dma_start(x_sb[:, k, :], xt_r[:, k, ds(start, w)])

                    aT = apool.tile([P, KH, 512], DT16, tag="a", name="aT")[:, :, :w]
                    for h in range(KH):
                        pg = psgu.tile([P, 512], F32, tag="psg", name="pg")[:, :w]
                        pu = psgu2.tile([P, 512], F32, tag="psu", name="pu")[:, :w]
                        if gu_interleave:
                            for k in range(KD):
                                nc.tensor.matmul(
                                    pg, wg_sb[:, k, h * P : (h + 1) * P], x_sb[:, k, :],
                                    start=(k == 0), stop=(k == KD - 1),
                                )
                                nc.tensor.matmul(
                                    pu, wu_sb[:, k, h * P : (h + 1) * P], x_sb[:, k, :],
                                    start=(k == 0), stop=(k == KD - 1),
                                )
                        else:
                            for k in range(KD):
                                nc.tensor.matmul(
                                    pg, wg_sb[:, k, h * P : (h + 1) * P], x_sb[:, k, :],
                                    start=(k == 0), stop=(k == KD - 1),
                                )
                            for k in range(KD):
                                nc.tensor.matmul(
                                    pu, wu_sb[:, k, h * P : (h + 1) * P], x_sb[:, k, :],
                                    start=(k == 0), stop=(k == KD - 1),
                                )
                        nc.scalar.activation(
                            aT[:, h, :], pg, mybir.ActivationFunctionType.Silu
                        )
                        nc.vector.tensor_tensor(
                            aT[:, h, :], aT[:, h, :], pu, mybir.AluOpType.mult
                        )

                    for g in range(w // P):
                        gg = (start + g * P) // P
                        out_sb = opool.tile([P, 4, 512], DT16 if y16 else F32, tag="o", name="out_sb")
                        for d4 in range(4):
                            py = psy.tile([P, 512], F32, tag="psy", name="py")
                            for h in range(KH):
                                nc.tensor.matmul(
                                    py,
                                    aT[:, h, g * P : (g + 1) * P],
                                    wd_sb[:, h, d4 * 512 : (d4 + 1) * 512],
                                    start=(h == 0),
                                    stop=(h == KH - 1),
                                )
                            nc.vector.tensor_scalar_mul(
                                out_sb[:, d4, :], py, cw_sb[:, gg : gg + 1]
                            )
                        se = nc.scalar if store_eng == "scalar" else nc.sync
                        if ysplit:
                            for d4 in range(4):
                                se.dma_start(
                                    y_r[:, gg, d4 * 512 : (d4 + 1) * 512],
                                    out_sb[:, d4, :],
                                )
                        else:
                            se.dma_start(y_r[:, gg, :], out_sb[:])

    nc.compile()
    return nc


def _route(x_flat, gate_w, expert_bias):
    """Replicate the reference router in numpy (fp32)."""
    N = x_flat.shape[0]
    logits = x_flat @ gate_w.T                       # [N, E]
    m = logits.max(-1, keepdims=True)
    p = np.exp(logits - m)
    p /= p.sum(-1, keepdims=True)
    biased = logits + expert_bias
    rows = np.arange(N)
    i1 = biased.argmax(-1)
    b2 = biased.copy()
    b2[rows, i1] = -np.inf
    i2 = b2.argmax(-1)
    w1 = p[rows, i1]
    w2 = p[rows, i2]
    s = w1 + w2
    return i1, i2, w1 / s, w2 / s


def _prepare(inputs):
    x = np.asarray(inputs["x"], dtype=np.float32)
    B, S_, D_ = x.shape
    assert D_ == D
    x_flat = x.reshape(-1, D)
    N = x_flat.shape[0]
    S0 = N // 8

    i1, i2, w1, w2 = _route(
        x_flat,
        np.asarray(inputs["gate_w"], dtype=np.float32),
        np.asarray(inputs["expert_bias"], dtype=np.float32),
    )

    idx_lists = []
    w_lists = []
    for e in range(E):
        m1 = i1 == e
        m2 = i2 == e
        idx = np.nonzero(m1 | m2)[0]
        w = np.where(m1[idx], w1[idx], w2[idx]).astype(np.float32)
        idx_lists.append(idx)
        w_lists.append(w)

    maxc = max(len(ix) for ix in idx_lists)
    C = ((maxc + P - 1) // P) * P
    TT = C + S0

    bf = np.float16
    Wg = np.asarray(inputs["Wg"], dtype=np.float32)
    Wu = np.asarray(inputs["Wu"], dtype=np.float32)
    Wd = np.asarray(inputs["Wd"], dtype=np.float32)
    wsg = np.ascontiguousarray(np.asarray(inputs["Ws_g"], np.float32).T).astype(bf)
    wsu = np.ascontiguousarray(np.asarray(inputs["Ws_u"], np.float32).T).astype(bf)
    wsd = np.ascontiguousarray(np.asarray(inputs["Ws_d"], np.float32).T).astype(bf)

    in_maps = []
    idx_pad = np.empty((E, C), dtype=np.int64)
    for e in range(E):
        idx = idx_lists[e]
        pad = np.full(C - len(idx), N, dtype=np.int64)  # N -> dummy row
        idx_pad[e] = np.concatenate([idx, pad])
        gather_idx = np.concatenate([idx, np.zeros(C - len(idx), np.int64)])

        xt = np.empty((D, TT), dtype=bf)
        xt[:, :C] = x_flat[gather_idx].T
        xt[:, C:] = x_flat[e * S0 : (e + 1) * S0].T

        cwv = np.ones(TT, dtype=np.float32)
        cwv[: len(idx)] = w_lists[e]
        cwv[len(idx) : C] = 0.0
        cwv = np.ascontiguousarray(cwv.reshape(TT // P, P).T)

        in_maps.append(
            {
                "xt": xt,
                "wg_e": np.ascontiguousarray(Wg[e].T).astype(bf),
                "wu_e": np.ascontiguousarray(Wu[e].T).astype(bf),
                "wd_e": np.ascontiguousarray(Wd[e].T).astype(bf),
                "wg_s": wsg,
                "wu_s": wsu,
                "wd_s": wsd,
                "cw": cwv,
            }
        )
    return x, in_maps, idx_pad, C, S0, N


def _combine(x_shape, results, idx_pad, C, S0, N):
    acc = np.zeros((N + 1, D), dtype=np.float32)
    for e in range(E):
        ye = results[e]["y"]
        acc[idx_pad[e]] += ye[:C]
        acc[e * S0 : (e + 1) * S0] += ye[C:]
    return acc[:N].reshape(x_shape)


def kernel(**inputs) -> np.ndarray:
    x, in_maps, idx_pad, C, S0, N = _prepare(inputs)
    nc = build_kernel(C, S0)
    res = run_bass_kernel_spmd(nc, in_maps, core_ids=list(range(8)))
    return _combine(x.shape, [res.results[e] for e in range(E)], idx_pad, C, S0, N)



# revision 29
# speedup vs baseline: 1.5357x; 1.1883x over previous
"""MoE layer (8 experts, top-2 routing + shared expert) on 8 Trainium2 cores.

Strategy (expert parallelism per the sharding hint):
  - Host computes the router (logits -> softmax -> top-2 -> combine weights)
    and *dispatches*: core e receives the tokens routed to expert e (gathered,
    transposed to [D, C] layout, fp16) plus a 1/8 data-parallel slice of all
    tokens for the shared expert.
  - Each core runs one Bass/Tile kernel computing, for its token set,
      y = (silu(x @ Wg.T) * (x @ Wu.T)) @ Wd.T   (scaled by combine weight)
    for its expert's weights, then the same with the shared-expert weights.
    All matmuls are fp16 with fp32 PSUM accumulation.
  - Host *combines*: scatter-adds the per-expert outputs and the shared
    outputs back into the full [N, D] result.

Device layout per core (SPMD, one NEFF):
  xt  [D, TT]  fp16   tokens on the free dim, D on partitions (16 k-tiles)
  wg,wu [D, H] fp16   expert-then-shared weight loads (H on free dim)
  wd  [H, D]   fp16
  cw  [128, TT/128] f32  per-token combine weight, pre-grouped on host so
                         the DMA is contiguous (1.0 for the shared slice)
  y   [TT, D]  f32    output, tokens on partitions at write time

Pipeline per 512-token chunk: 2*11*16 matmuls produce g,u in PSUM per
128-row H tile; ScalarE applies Silu, VectorE multiplies into an fp16 act
tile [H, chunk]; 4x4x11 matmuls then contract act.T @ WdT into [128 tokens,
512 D] PSUM tiles, which VectorE scales by cw and DMAs out.
"""

import numpy as np

import concourse.mybir as mybir
import concourse.tile as tile
from concourse import bacc
from concourse.bass import ds
from concourse.bass_utils import run_bass_kernel_spmd

P = 128
D = 2048
H = 1408
E = 8
TOP_K = 2
KD = D // P   # 16
KH = H // P   # 11
DT16 = mybir.dt.float16  # fp16: same PE rate as bf16, 8x the mantissa precision
F32 = mybir.dt.float32


def _chunks(count, base):
    """Split `count` tokens (multiple of 128) into chunks of 512 then 128."""
    out = []
    pos = 0
    while count - pos >= 512:
        out.append((base + pos, 512))
        pos += 512
    while count - pos >= P:
        out.append((base + pos, P))
        pos += P
    assert pos == count
    return out


def build_kernel(C, S, repeat=1, xb=2, ab=3, ob=2, pgu=3, pyb=2, wd_late=False, ysplit=False, tail_first=False, psg=None, psu=None, bulk_dma=True, store_eng="sync", y16=True, hsplit=4, gu_interleave=True, wd_eng="sync", x_eng="sync", delay=1):
    """Build the SPMD Bass module for C expert tokens + S shared tokens."""
    TT = C + S
    assert C % P == 0 and S % P == 0

    nc = bacc.Bacc(
        "TRN2",
        target_bir_lowering=False,
        debug=False,
        enable_asserts=False,
        num_devices=8,
    )

    xt = nc.dram_tensor("xt", [D, TT], DT16, kind="ExternalInput").ap()
    wts = {}
    for pref in ("e", "s"):
        wts[pref] = (
            nc.dram_tensor(f"wg_{pref}", [D, H], DT16, kind="ExternalInput").ap(),
            nc.dram_tensor(f"wu_{pref}", [D, H], DT16, kind="ExternalInput").ap(),
            nc.dram_tensor(f"wd_{pref}", [H, D], DT16, kind="ExternalInput").ap(),
        )
    cw = nc.dram_tensor("cw", [P, TT // P], F32, kind="ExternalInput").ap()
    y = nc.dram_tensor("y", [TT, D], DT16 if y16 else F32, kind="ExternalOutput").ap()

    xt_r = xt.rearrange("(ko p) t -> p ko t", p=P)     # [128, 16, TT]
    y_r = y.rearrange("(g p) d -> p g d", p=P)         # [128, TT/128, 2048]
    cw_r = cw  # already [128, TT/128] host-transposed

    phases = [("e", 0, C), ("s", C, S)]

    with tile.TileContext(nc) as tc:
        with (
            tc.tile_pool(name="wgp", bufs=1) as wgp,
            tc.tile_pool(name="wup", bufs=1) as wup,
            tc.tile_pool(name="wdp", bufs=1) as wdp,
            tc.tile_pool(name="xp", bufs=xb) as xp,
            tc.tile_pool(name="ap", bufs=ab) as apool,
            tc.tile_pool(name="op", bufs=ob) as opool,
            tc.tile_pool(name="cp", bufs=1) as cpool,
            tc.tile_pool(name="psgu", bufs=pgu, space="PSUM") as psgu,
            tc.tile_pool(name="psgu2", bufs=(psu or pgu), space="PSUM") as psgu2,
            tc.tile_pool(name="psy", bufs=pyb, space="PSUM") as psy,
        ):
            cw_sb = cpool.tile([P, TT // P], F32)
            nc.sync.dma_start(cw_sb[:], cw_r)

            pend = []
            for pref, base, count in phases * repeat:
                if count == 0:
                    continue
                wg_d, wu_d, wd_d = wts[pref]
                wg_sb = wgp.tile([P, KD, H], DT16, tag="wg")
                wu_sb = wup.tile([P, KD, H], DT16, tag="wu")
                wg_rr = wg_d.rearrange("(ko p) h -> p ko h", p=P)
                wu_rr = wu_d.rearrange("(ko p) h -> p ko h", p=P)
                chunk_list = _chunks(count, base)
                if tail_first:
                    chunk_list = chunk_list[::-1]
                # bulk DMAs: a single large dma_start fans out across several
                # HW-DGE queues on real hardware (measured ~120us faster than
                # k-tile-split DMAs, even though the cost model says otherwise)
                start0, w0 = chunk_list[0]
                x0_sb = xp.tile([P, KD, 512], DT16, tag="x", name="x0_sb")[:, :, :w0]
                if bulk_dma:
                    nc.sync.dma_start(x0_sb[:], xt_r[:, :, ds(start0, w0)])
                    if hsplit > 1:
                        # split along H so early h-tiles' weights land first;
                        # pieces stay >=1.4MB for multi-queue DMA fanout
                        bounds = [H * i // hsplit for i in range(hsplit + 1)]
                        bounds = [(b // P) * P for b in bounds]
                        bounds[-1] = H
                        for h0, h1 in zip(bounds[:-1], bounds[1:]):
                            nc.sync.dma_start(wg_sb[:, :, h0:h1], wg_rr[:, :, h0:h1])
                            nc.sync.dma_start(wu_sb[:, :, h0:h1], wu_rr[:, :, h0:h1])
                    else:
                        nc.sync.dma_start(wg_sb[:], wg_rr)
                        nc.sync.dma_start(wu_sb[:], wu_rr)
                else:
                    for k in range(KD):
                        nc.sync.dma_start(x0_sb[:, k, :], xt_r[:, k, ds(start0, w0)])
                        nc.sync.dma_start(wg_sb[:, k, :], wg_rr[:, k, :])
                        nc.sync.dma_start(wu_sb[:, k, :], wu_rr[:, k, :])
                wd_sb = wdp.tile([P, KH, D], DT16, tag="wd")
                wd_rr = wd_d.rearrange("(ho p) d -> p ho d", p=P)
                wde = nc.gpsimd if wd_eng == "pool" else nc.sync
                if bulk_dma:
                    wde.dma_start(wd_sb[:], wd_rr)
                elif not wd_late:
                    for h in range(KH):
                        nc.sync.dma_start(wd_sb[:, h, :], wd_rr[:, h, :])

                for ci, (start, w) in enumerate(chunk_list):
                    if wd_late and ci == 1:
                        for h in range(KH):
                            nc.sync.dma_start(wd_sb[:, h, :], wd_rr[:, h, :])
                    if ci == 0:
                        x_sb = x0_sb
                    else:
                        x_sb = xp.tile([P, KD, 512], DT16, tag="x", name="x_sb")[:, :, :w]
                        xe = nc.gpsimd if x_eng == "pool" else nc.sync
                        if bulk_dma:
                            xe.dma_start(x_sb, xt_r[:, :, ds(start, w)])
                        else:
                            for k in range(KD):
                                nc.sync.dma_start(x_sb[:, k, :], xt_r[:, k, ds(start, w)])

                    aT = apool.tile([P, KH, 512], DT16, tag="a", name="aT")[:, :, :w]
                    for h in range(KH):
                        pg = psgu.tile([P, 512], F32, tag="psg", name="pg")[:, :w]
                        pu = psgu2.tile([P, 512], F32, tag="psu", name="pu")[:, :w]
                        if gu_interleave:
                            for k in range(KD):
                                nc.tensor.matmul(
                                    pg, wg_sb[:, k, h * P : (h + 1) * P], x_sb[:, k, :],
                                    start=(k == 0), stop=(k == KD - 1),
                                )
                                nc.tensor.matmul(
                                    pu, wu_sb[:, k, h * P : (h + 1) * P], x_sb[:, k, :],
                                    start=(k == 0), stop=(k == KD - 1),
                                )
                        else:
                            for k in range(KD):
                                nc.tensor.matmul(
                                    pg, wg_sb[:, k, h * P : (h + 1) * P], x_sb[:, k, :],
                                    start=(k == 0), stop=(k == KD - 1),
                                )
                            for k in range(KD):
                                nc.tensor.matmul(
                                    pu, wu_sb[:, k, h * P : (h + 1) * P], x_sb[:, k, :],
                                    start=(k == 0), stop=(k == KD - 1),
                                )
                        nc.scalar.activation(
                            aT[:, h, :], pg, mybir.ActivationFunctionType.Silu
                        )
                        nc.vector.tensor_tensor(
                            aT[:, h, :], aT[:, h, :], pu, mybir.AluOpType.mult
                        )

                    def emit_g2(start, w, aT, wd_sb=wd_sb):
                        for g in range(w // P):
                            gg = (start + g * P) // P
                            out_sb = opool.tile([P, 4, 512], DT16 if y16 else F32, tag="o", name="out_sb")
                            for d4 in range(4):
                                py = psy.tile([P, 512], F32, tag="psy", name="py")
                                for h in range(KH):
                                    nc.tensor.matmul(
                                        py,
                                        aT[:, h, g * P : (g + 1) * P],
                                        wd_sb[:, h, d4 * 512 : (d4 + 1) * 512],
                                        start=(h == 0),
                                        stop=(h == KH - 1),
                                    )
                                nc.vector.tensor_scalar_mul(
                                    out_sb[:, d4, :], py, cw_sb[:, gg : gg + 1]
                                )
                            se = nc.scalar if store_eng == "scalar" else nc.sync
                            if ysplit:
                                for d4 in range(4):
                                    se.dma_start(
                                        y_r[:, gg, d4 * 512 : (d4 + 1) * 512],
                                        out_sb[:, d4, :],
                                    )
                            else:
                                se.dma_start(y_r[:, gg, :], out_sb[:])

                    if delay:
                        # software pipeline: gemm2 of an earlier chunk runs on
                        # the PE while ACT/DVE finish producing aT of chunk c
                        if len(pend) >= delay:
                            emit_g2(*pend.pop(0))
                        pend.append((start, w, aT))
                    else:
                        emit_g2(start, w, aT)
                # drain before the next phase's weight DMAs overwrite wd_sb
                while pend:
                    emit_g2(*pend.pop(0))

    nc.compile()
    return nc


def _route(x_flat, gate_w, expert_bias):
    """Replicate the reference router in numpy (fp32)."""
    N = x_flat.shape[0]
    logits = x_flat @ gate_w.T                       # [N, E]
    m = logits.max(-1, keepdims=True)
    p = np.exp(logits - m)
    p /= p.sum(-1, keepdims=True)
    biased = logits + expert_bias
    rows = np.arange(N)
    i1 = biased.argmax(-1)
    b2 = biased.copy()
    b2[rows, i1] = -np.inf
    i2 = b2.argmax(-1)
    w1 = p[rows, i1]
    w2 = p[rows, i2]
    s = w1 + w2
    return i1, i2, w1 / s, w2 / s


def _prepare(inputs):
    x = np.asarray(inputs["x"], dtype=np.float32)
    B, S_, D_ = x.shape
    assert D_ == D
    x_flat = x.reshape(-1, D)
    N = x_flat.shape[0]
    S0 = N // 8

    i1, i2, w1, w2 = _route(
        x_flat,
        np.asarray(inputs["gate_w"], dtype=np.float32),
        np.asarray(inputs["expert_bias"], dtype=np.float32),
    )

    idx_lists = []
    w_lists = []
    for e in range(E):
        m1 = i1 == e
        m2 = i2 == e
        idx = np.nonzero(m1 | m2)[0]
        w = np.where(m1[idx], w1[idx], w2[idx]).astype(np.float32)
        idx_lists.append(idx)
        w_lists.append(w)

    maxc = max(len(ix) for ix in idx_lists)
    C = ((maxc + P - 1) // P) * P
    TT = C + S0

    bf = np.float16
    Wg = np.asarray(inputs["Wg"], dtype=np.float32)
    Wu = np.asarray(inputs["Wu"], dtype=np.float32)
    Wd = np.asarray(inputs["Wd"], dtype=np.float32)
    wsg = np.ascontiguousarray(np.asarray(inputs["Ws_g"], np.float32).T).astype(bf)
    wsu = np.ascontiguousarray(np.asarray(inputs["Ws_u"], np.float32).T).astype(bf)
    wsd = np.ascontiguousarray(np.asarray(inputs["Ws_d"], np.float32).T).astype(bf)

    in_maps = []
    idx_pad = np.empty((E, C), dtype=np.int64)
    for e in range(E):
        idx = idx_lists[e]
        pad = np.full(C - len(idx), N, dtype=np.int64)  # N -> dummy row
        idx_pad[e] = np.concatenate([idx, pad])
        gather_idx = np.concatenate([idx, np.zeros(C - len(idx), np.int64)])

        xt = np.empty((D, TT), dtype=bf)
        xt[:, :C] = x_flat[gather_idx].T
        xt[:, C:] = x_flat[e * S0 : (e + 1) * S0].T

        cwv = np.ones(TT, dtype=np.float32)
        cwv[: len(idx)] = w_lists[e]
        cwv[len(idx) : C] = 0.0
        cwv = np.ascontiguousarray(cwv.reshape(TT // P, P).T)

        in_maps.append(
            {
                "xt": xt,
                "wg_e": np.ascontiguousarray(Wg[e].T).astype(bf),
                "wu_e": np.ascontiguousarray(Wu[e].T).astype(bf),
                "wd_e": np.ascontiguousarray(Wd[e].T).astype(bf),
                "wg_s": wsg,
                "wu_s": wsu,
                "wd_s": wsd,
                "cw": cwv,
            }
        )
    return x, in_maps, idx_pad, C, S0, N


def _combine(x_shape, results, idx_pad, C, S0, N):
    acc = np.zeros((N + 1, D), dtype=np.float32)
    for e in range(E):
        ye = results[e]["y"]
        acc[idx_pad[e]] += ye[:C]
        acc[e * S0 : (e + 1) * S0] += ye[C:]
    return acc[:N].reshape(x_shape)


def kernel(**inputs) -> np.ndarray:
    x, in_maps, idx_pad, C, S0, N = _prepare(inputs)
    nc = build_kernel(C, S0)
    res = run_bass_kernel_spmd(nc, in_maps, core_ids=list(range(8)))
    return _combine(x.shape, [res.results[e] for e in range(E)], idx_pad, C, S0, N)

